# revision 32
# baseline (speedup 1.0000x reference)
"""Trainium2 Bass kernel for a ViT-style transformer block (nn_Block_11132555231612).

Data-parallel over batch across 8 NeuronCores (2 sequences of 1024 tokens per
core). fp8e4m3 DoubleRow matmuls (0.5 cyc/row, 256-deep contraction) carry
QKV / AV / proj / fc1 / fc2; scores stay bf16 (64-deep contraction). The
residual stream is held at 32x scale (x scaled on host, weights scaled to
match, output unscaled on host) so every fp8 weight scale folds away without
device fixups; softmax normalization keeps the appended-ones-column trick
(column value = 32 cancels the V scale). fc1/fc2 weights use hi+lo double-fp8
planes accumulated in PSUM to recover near-bf16 weight precision. LN applies
run on the (otherwise idle) GPSIMD engine to keep the scalar engine free for
softmax exp, which is the critical path of the attention phase.
"""

import os
import sys

sys.path.insert(0, "/opt/trn_rl_repo")

import numpy as np
import ml_dtypes

import concourse.bass as bass
import concourse.mybir as mybir
import concourse.tile as tile
from concourse import bacc
from concourse.bass_utils import run_bass_kernel_spmd
from concourse.masks import make_identity
from contextlib import ExitStack

F32 = mybir.dt.float32
BF16 = mybir.dt.bfloat16
FP8 = mybir.dt.float8e4
NP8 = ml_dtypes.float8_e4m3
AF = mybir.ActivationFunctionType
DR = mybir.MatmulPerfMode.DoubleRow
ALU = mybir.AluOpType

P = 128
B_PER_CORE = 2
SEQ = 1024
T = B_PER_CORE * SEQ          # 2048 tokens per core
C = 768
H = 12
HD = 64
HID = 3072
KS = C // P                   # 6
HS = HID // P                 # 24
NT = T // P                   # 16 token tiles
EPS = 1e-5
SCALE = HD ** -0.5            # 0.125
S = 32.0                      # residual / weight scale
VP = 80                       # padded V row (65 used): 16B dual-fp8 ldweights

_CACHED_NC = None


class TileKernel:
    b1_zero = False
    bv_zero = False
    bproj_zero = False
    b2_zero = False
    bqk_zero = False
    w1x2 = True
    w2x2 = True

    def __init__(self, nc):
        self.nc = nc
        self.stack = ExitStack()
        self.tc = None

    def __enter__(self):
        self.tc = self.stack.enter_context(tile.TileContext(self.nc))
        return self

    def __exit__(self, *exc):
        return self.stack.__exit__(*exc)

    def ln_tile(self, xt, dst, dst_col, work, psum_ln, eps_t, ident):
        """LN of one token-major tile xt [P, C] -> feature-major dst tile
        columns [P, KS, P] at dst[:, :, dst_col:dst_col+P].

        Stats on DVE, apply on GPSIMD (tensor_scalar), transpose on PE in
        bf16 (hw rejects fp8 transposes), psum->sbuf copy converts dtype.
        """
        nc = self.nc
        st = work.tile([P, 2, 6], F32, tag="bnstats")
        xg = xt.rearrange("p (s d) -> p s d", s=2)
        for s in range(2):
            nc.vector.bn_stats(st[:, s, :], xg[:, s, :])
        mv = work.tile([P, 2], F32, tag="mv")
        nc.vector.bn_aggr(mv[:], st[:])
        sdv = work.tile([P, 1], F32, tag="sdv")
        nc.scalar.activation(sdv[:], mv[:, 1:2], AF.Sqrt, bias=eps_t[:])
        rstd = work.tile([P, 1], F32, tag="rstd")
        nc.vector.reciprocal(rstd[:], sdv[:])
        nmu = work.tile([P, 1], F32, tag="nmu")
        nc.vector.tensor_scalar_mul(nmu[:], mv[:, 0:1], -1.0)
        xn = work.tile([P, C], BF16, tag="xn")
        nc.gpsimd.tensor_scalar(xn[:], xt, nmu[:], rstd[:],
                                op0=ALU.add, op1=ALU.mult)
        pt = psum_ln.tile([P, KS, P], BF16, tag="tp")
        for c in range(KS):
            nc.tensor.transpose(pt[:, c, :], xn[:, c * P:(c + 1) * P],
                                ident[:])
        nc.any.tensor_copy(dst[:, :, dst_col:dst_col + P], pt[:])

    def run(self, x_d, out_d, wqkv_d, bqkv_d, bv_d, wproj_d, bproj_d,
            w1h_d, w1l_d, b1_d, w2h_d, w2l_d, b2_d):
        nc, tc, S_ = self.nc, self.tc, self.stack
        const = S_.enter_context(tc.tile_pool(name="const", bufs=1))
        xpool = S_.enter_context(tc.tile_pool(name="xres", bufs=1))
        work = S_.enter_context(tc.tile_pool(name="work", bufs=5))

        ident16 = const.tile([P, P], BF16)
        make_identity(nc, ident16[:])
        eps_t = const.tile([P, 1], F32)
        nc.vector.memset(eps_t[:], EPS * S * S)
        if not self.bqk_zero:
            bqkv_sb = const.tile([P, 12], F32)
            nc.sync.dma_start(bqkv_sb[:], bqkv_d[:])
        if not self.b1_zero:
            b1_sb = const.tile([P, HS], F32)
            nc.sync.dma_start(b1_sb[:], b1_d[:])
        x_sb = xpool.tile([P, NT, C], F32)
        xr = x_d[:].rearrange("(n p) c -> p n c", p=P)
        for t4 in range(8):
            nc.sync.dma_start(x_sb[:, t4 * 2:(t4 + 1) * 2, :],
                              xr[:, t4 * 2:(t4 + 1) * 2, :])

        ablate = os.environ.get("TRN_ABLATE", "")
        # oT / wproj live until proj; everything else attention-local frees
        # before the MLP weights arrive.
        o_p = S_.enter_context(tc.tile_pool(name="oT", bufs=1))
        wp_p = S_.enter_context(tc.tile_pool(name="wpp", bufs=1))
        oT = o_p.tile([P, KS, T], FP8)
        if ablate != "skip_attn":
         with ExitStack() as attn_win:
            wproj_sb = wp_p.tile([P, KS, C], FP8)
            nc.sync.dma_start(wproj_sb[:], wproj_d[:])
            qkT_p = attn_win.enter_context(tc.tile_pool(name="qkT", bufs=1))
            v_p = attn_win.enter_context(tc.tile_pool(name="vtile", bufs=1))
            wq_p = attn_win.enter_context(tc.tile_pool(name="wqp", bufs=1))
            qkT = qkT_p.tile([P, 12, T], BF16)
            V_sb = v_p.tile([P, NT, H, VP], FP8)
            wqkv_sb = wq_p.tile([P, KS, 3 * C], FP8)
            for c2 in range(3):
                nc.sync.dma_start(wqkv_sb[:, c2 * 2:c2 * 2 + 2, :],
                                  wqkv_d[:, c2 * 2:c2 * 2 + 2, :])

            with ExitStack() as s1:
                xnT_p = s1.enter_context(tc.tile_pool(name="xnT1", bufs=1))
                psum_ln = s1.enter_context(
                    tc.tile_pool(name="psln", bufs=2, space="PSUM"))
                psum_mm = s1.enter_context(
                    tc.tile_pool(name="psmm", bufs=2, space="PSUM"))
                bv_p = s1.enter_context(tc.tile_pool(name="bvp", bufs=1))

                if not self.bv_zero:
                    bv_bc = bv_p.tile([P, C], F32)
                    nc.sync.dma_start(bv_bc[:],
                                      bv_d[:].partition_broadcast(P))

                xnT = xnT_p.tile([P, KS, T], FP8)
                for t in range(NT):
                    self.ln_tile(x_sb[:, t, :], xnT, t * P, work, psum_ln,
                                 eps_t, ident16)

                # V token-major with S-valued column at slot 64 (denominator
                # trick: cancels the S scale of V on normalization)
                nc.vector.memset(V_sb[:, :, :, HD], S)
                for t in range(NT):
                    psv = psum_mm.tile([P, C], F32, tag="psv")
                    for (n0, nsz) in ((0, 512), (512, 256)):
                        for c2 in range(3):
                            nc.tensor.matmul(
                                psv[:, n0:n0 + nsz],
                                xnT[:, c2 * 2:c2 * 2 + 2, t * P:(t + 1) * P],
                                wqkv_sb[:, c2 * 2:c2 * 2 + 2,
                                        2 * C + n0:2 * C + n0 + nsz],
                                start=(c2 == 0), stop=(c2 == 2),
                                perf_mode=DR)
                    if self.bv_zero:
                        nc.any.tensor_copy(
                            V_sb[:, t, :, 0:HD],
                            psv[:].rearrange("p (h d) -> p h d", h=H))
                    else:
                        nc.vector.tensor_add(
                            V_sb[:, t, :, 0:HD],
                            psv[:].rearrange("p (h d) -> p h d", h=H),
                            bv_bc[:].rearrange("p (h d) -> p h d", h=H))

                # q^T / k^T feature-major, head-pair order (q then k per pair)
                for oct in [x for p_ in range(6) for x in (p_, 6 + p_)]:
                    for nch in range(T // 512):
                        ps = psum_mm.tile([P, 512], F32, tag="ps")
                        for c2 in range(3):
                            nc.tensor.matmul(
                                ps[:],
                                wqkv_sb[:, c2 * 2:c2 * 2 + 2,
                                        oct * P:(oct + 1) * P],
                                xnT[:, c2 * 2:c2 * 2 + 2,
                                    nch * 512:(nch + 1) * 512],
                                start=(c2 == 0), stop=(c2 == 2),
                                perf_mode=DR)
                        if self.bqk_zero:
                            nc.any.tensor_copy(
                                qkT[:, oct, nch * 512:(nch + 1) * 512], ps[:])
                        else:
                            nc.vector.tensor_scalar_add(
                                qkT[:, oct, nch * 512:(nch + 1) * 512], ps[:],
                                bqkv_sb[:, oct:oct + 1])

            # ---- attention ----
            with ExitStack() as s2:
                psum_s = s2.enter_context(
                    tc.tile_pool(name="pss", bufs=2, space="PSUM"))
                psum_o = s2.enter_context(
                    tc.tile_pool(name="pso", bufs=2, space="PSUM"))
                awork = s2.enter_context(tc.tile_pool(name="awork", bufs=3))
                for b in range(B_PER_CORE):
                    for h in range(H):
                        po = (h % 2) * 64
                        oq, ok = h // 2, 6 + h // 2
                        for qc in range(SEQ // 512):
                            qs = b * SEQ + qc * 512
                            pso = psum_o.tile([P, 512], F32, tag="pso")
                            kt0 = 0
                            for g in (3, 3, 2):
                                pss = psum_s.tile([P, 3, 512], F32,
                                                  tag="pss")
                                for j in range(g):
                                    ko = b * SEQ + (kt0 + j) * P
                                    nc.tensor.matmul(
                                        pss[:, j, :],
                                        qkT[po:po + HD, ok, ko:ko + P],
                                        qkT[po:po + HD, oq, qs:qs + 512],
                                        start=True, stop=True)
                                pr = awork.tile([P, 3, 512], FP8,
                                                tag="probs")
                                nc.scalar.activation(
                                    pr[:, 0:g, :].rearrange(
                                        "p a b -> p (a b)"),
                                    pss[:, 0:g, :].rearrange(
                                        "p a b -> p (a b)"),
                                    AF.Exp, scale=SCALE / (S * S))
                                vb = b * 8 + kt0
                                last = kt0 + g == 8
                                nc.tensor.matmul(
                                    pso[0:HD + 1, :],
                                    V_sb[:, vb:vb + 2, h, 0:HD + 1],
                                    pr[:, 0:2, :],
                                    start=(kt0 == 0),
                                    stop=(last and g == 2),
                                    perf_mode=DR)
                                if g == 3:
                                    nc.tensor.matmul(
                                        pso[0:HD + 1, :],
                                        V_sb[:, vb + 2, h, 0:HD + 1],
                                        pr[:, 2, :],
                                        start=False, stop=last)
                                kt0 += g
                            rc = awork.tile([P, 512], F32, tag="recip")
                            nc.vector.reciprocal(rc[HD:HD + 1, :],
                                                 pso[HD:HD + 1, :])
                            rc0 = awork.tile([1, 512], F32, tag="rc0")
                            nc.sync.dma_start(rc0[:], rc[HD:HD + 1, :])
                            rbc = awork.tile([HD, 512], F32, tag="rbc")
                            nc.gpsimd.partition_broadcast(
                                rbc[:], rc0[0:1, :], channels=HD)
                            if h % 2 == 0:
                                nc.vector.tensor_mul(
                                    oT[0:HD, h // 2, qs:qs + 512],
                                    pso[0:HD, :], rbc[:])
                            else:
                                osc = awork.tile([HD, 512], FP8, tag="osc")
                                nc.vector.tensor_mul(osc[:], pso[0:HD, :],
                                                     rbc[:])
                                nc.sync.dma_start(
                                    oT[64:128, h // 2, qs:qs + 512], osc[:])

        # ---- MLP weights (DMA overlaps proj/LN2), proj + LN2, MLP ----
        if ablate != "skip_mlp":
         with ExitStack() as s4:
            w_p = s4.enter_context(tc.tile_pool(name="wmlp", bufs=1))
            xnT_p2 = s4.enter_context(tc.tile_pool(name="xnT2", bufs=1))
            h_p = s4.enter_context(tc.tile_pool(name="hT", bufs=2))
            b2_p = s4.enter_context(tc.tile_pool(name="b2p", bufs=1))

            w1_planes = []
            w1h_sb = w_p.tile([P, KS, HID], FP8)
            for q in range(8):
                nc.sync.dma_start(w1h_sb[:, :, q * 384:(q + 1) * 384],
                                  w1h_d[:, :, q * 384:(q + 1) * 384])
            w1_planes.append(w1h_sb)
            if self.w1x2:
                w1l_sb = w_p.tile([P, KS, HID], FP8)
                for q in range(8):
                    nc.sync.dma_start(w1l_sb[:, :, q * 384:(q + 1) * 384],
                                      w1l_d[:, :, q * 384:(q + 1) * 384])
                w1_planes.append(w1l_sb)
            w2_planes = []
            w2h_sb = w_p.tile([P, HS, C], FP8)
            for c3 in range(4):
                nc.sync.dma_start(w2h_sb[:, c3 * 6:c3 * 6 + 6, :],
                                  w2h_d[:, c3 * 6:c3 * 6 + 6, :])
            w2_planes.append(w2h_sb)
            if self.w2x2:
                w2l_sb = w_p.tile([P, HS, C], FP8)
                for c3 in range(4):
                    nc.sync.dma_start(w2l_sb[:, c3 * 6:c3 * 6 + 6, :],
                                      w2l_d[:, c3 * 6:c3 * 6 + 6, :])
                w2_planes.append(w2l_sb)

            if not self.b2_zero:
                b2_bc = b2_p.tile([P, C], F32)
                nc.sync.dma_start(b2_bc[:], b2_d[:].partition_broadcast(P))

            xnT2 = xnT_p2.tile([P, KS, T], FP8)
            # proj + residual + LN2, interleaved per token tile
            with ExitStack() as s3:
                psum_p = s3.enter_context(
                    tc.tile_pool(name="psp", bufs=2, space="PSUM"))
                psum_ln2 = s3.enter_context(
                    tc.tile_pool(name="psln2", bufs=2, space="PSUM"))
                bp_p = s3.enter_context(tc.tile_pool(name="bpp", bufs=1))
                if not self.bproj_zero:
                    bproj_bc = bp_p.tile([P, C], F32)
                    nc.sync.dma_start(bproj_bc[:],
                                      bproj_d[:].partition_broadcast(P))
                for t in range(NT):
                    if ablate != "skip_attn":
                        psp = psum_p.tile([P, C], F32, tag="psp")
                        for (n0, nsz) in ((0, 512), (512, 256)):
                            for c2 in range(3):
                                nc.tensor.matmul(
                                    psp[:, n0:n0 + nsz],
                                    oT[:, c2 * 2:c2 * 2 + 2, t * P:(t + 1) * P],
                                    wproj_sb[:, c2 * 2:c2 * 2 + 2, n0:n0 + nsz],
                                    start=(c2 == 0), stop=(c2 == 2),
                                    perf_mode=DR)
                        nc.vector.tensor_add(x_sb[:, t, :], x_sb[:, t, :],
                                             psp[:])
                        if not self.bproj_zero:
                            nc.vector.tensor_add(x_sb[:, t, :], x_sb[:, t, :],
                                                 bproj_bc[:])
                    self.ln_tile(x_sb[:, t, :], xnT2, t * P, work, psum_ln2,
                                 eps_t, ident16)

            psum_1 = s4.enter_context(
                tc.tile_pool(name="ps1", bufs=2, space="PSUM"))
            psum_2 = s4.enter_context(
                tc.tile_pool(name="ps2", bufs=2, space="PSUM"))

            n1 = 3 * len(w1_planes)
            n2 = 12 * len(w2_planes)
            for tq in range(T // 512):
                t0 = tq * 512
                hT = h_p.tile([P, HS, 512], FP8, tag="hT")
                for hp in range(12):               # hidden-feature pairs
                    ps1 = psum_1.tile([P, 2, 512], F32, tag="ps1")
                    for j in range(2):
                        i = 0
                        for w1p in w1_planes:
                            for c3 in range(3):
                                nc.tensor.matmul(
                                    ps1[:, j, :],
                                    w1p[:, c3 * 2:c3 * 2 + 2,
                                        (hp * 2 + j) * P:(hp * 2 + j + 1) * P],
                                    xnT2[:, c3 * 2:c3 * 2 + 2, t0:t0 + 512],
                                    start=(i == 0), stop=(i == n1 - 1),
                                    perf_mode=DR)
                                i += 1
                    if self.b1_zero:
                        nc.scalar.activation(
                            hT[:, hp * 2:hp * 2 + 2, :].rearrange(
                                "p a b -> p (a b)"),
                            ps1[:].rearrange("p a b -> p (a b)"),
                            AF.Gelu, scale=1.0 / S)
                    else:
                        for j in range(2):
                            nc.scalar.activation(
                                hT[:, hp * 2 + j, :], ps1[:, j, :],
                                AF.Gelu,
                                bias=b1_sb[:, hp * 2 + j:hp * 2 + j + 1],
                                scale=1.0 / S)
                for tt in range(4):
                    tg = tq * 4 + tt
                    ps2 = psum_2.tile([P, C], F32, tag="ps2")
                    for (n0, nsz) in ((0, 512), (512, 256)):
                        i = 0
                        for hp in range(12):
                            for w2p in w2_planes:
                                nc.tensor.matmul(
                                    ps2[:, n0:n0 + nsz],
                                    hT[:, hp * 2:hp * 2 + 2,
                                       tt * P:(tt + 1) * P],
                                    w2p[:, hp * 2:hp * 2 + 2, n0:n0 + nsz],
                                    start=(i == 0), stop=(i == n2 - 1),
                                    perf_mode=DR)
                                i += 1
                    nc.vector.tensor_add(x_sb[:, tg, :], x_sb[:, tg, :],
                                         ps2[:])
                    if not self.b2_zero:
                        nc.vector.tensor_add(x_sb[:, tg, :], x_sb[:, tg, :],
                                             b2_bc[:])
                nc.sync.dma_start(
                    out_d[:].rearrange("(n p) c -> p n c", p=P)[:, tq * 4:tq * 4 + 4, :],
                    x_sb[:, tq * 4:tq * 4 + 4, :])


def _build(b1_zero=False, bv_zero=False, bproj_zero=False, b2_zero=False,
           bqk_zero=False, w1x2=True, w2x2=True):
    nc = bacc.Bacc(None, target_bir_lowering=False, debug=False)

    x_d = nc.dram_tensor("x", [T, C], F32, kind="ExternalInput")
    out_d = nc.dram_tensor("out", [T, C], F32, kind="ExternalOutput")
    wqkv_d = nc.dram_tensor("wqkv", [P, KS, 3 * C], FP8, kind="ExternalInput")
    bqkv_d = nc.dram_tensor("bqkv", [P, 12], F32, kind="ExternalInput")
    bv_d = nc.dram_tensor("bv", [C], F32, kind="ExternalInput")
    wproj_d = nc.dram_tensor("wproj", [P, KS, C], FP8, kind="ExternalInput")
    bproj_d = nc.dram_tensor("bproj", [C], F32, kind="ExternalInput")
    w1h_d = nc.dram_tensor("w1h", [P, KS, HID], FP8, kind="ExternalInput")
    w1l_d = nc.dram_tensor("w1l", [P, KS, HID], FP8, kind="ExternalInput")
    b1_d = nc.dram_tensor("b1", [P, HS], F32, kind="ExternalInput")
    w2h_d = nc.dram_tensor("w2h", [P, HS, C], FP8, kind="ExternalInput")
    w2l_d = nc.dram_tensor("w2l", [P, HS, C], FP8, kind="ExternalInput")
    b2_d = nc.dram_tensor("b2", [C], F32, kind="ExternalInput")
    with TileKernel(nc) as tk:
        tk.b1_zero = b1_zero
        tk.bqk_zero = bqk_zero
        tk.bv_zero = bv_zero
        tk.bproj_zero = bproj_zero
        tk.b2_zero = b2_zero
        tk.w1x2 = w1x2
        tk.w2x2 = w2x2
        tk.run(x_d, out_d, wqkv_d, bqkv_d, bv_d, wproj_d, bproj_d,
               w1h_d, w1l_d, b1_d, w2h_d, w2l_d, b2_d)

    nc.compile()
    return nc


def _hilo(w):
    hi = w.astype(NP8)
    lo = (w - hi.astype(np.float32)).astype(NP8)
    return hi, lo


def _prep_host(inputs):
    f = lambda a: np.asarray(a, dtype=np.float32)
    x = f(inputs["x"])
    ln1_g, ln1_b = f(inputs["ln1_g"]), f(inputs["ln1_b"])
    ln2_g, ln2_b = f(inputs["ln2_g"]), f(inputs["ln2_b"])
    qkv_w = f(inputs["qkv_w"])
    proj_w, proj_b = f(inputs["proj_w"]), f(inputs["proj_b"])
    fc1_w, fc1_b = f(inputs["fc1_w"]), f(inputs["fc1_b"])
    fc2_w, fc2_b = f(inputs["fc2_w"]), f(inputs["fc2_b"])

    wqkv = np.ascontiguousarray(
        (qkv_w * ln1_g[None, :] * S).T.reshape(KS, P, 3 * C).transpose(1, 0, 2)
    ).astype(NP8)
    bqkv_full = S * (qkv_w @ ln1_b)                # [2304], S-scaled
    bqkv = np.ascontiguousarray(bqkv_full[:2 * C].reshape(12, P).T)
    bv = np.ascontiguousarray(bqkv_full[2 * C:])
    wproj = np.ascontiguousarray(
        (proj_w * S).T.reshape(KS, P, C).transpose(1, 0, 2)).astype(NP8)
    w1 = np.ascontiguousarray(
        (fc1_w * ln2_g[None, :] * S).T.reshape(KS, P, HID).transpose(1, 0, 2))
    w1h, w1l = _hilo(w1)
    b1 = np.ascontiguousarray((fc1_b + fc1_w @ ln2_b).reshape(HS, P).T)
    w2 = np.ascontiguousarray(
        (fc2_w * S).T.reshape(HS, P, C).transpose(1, 0, 2))
    w2h, w2l = _hilo(w2)

    shared = {
        "wqkv": wqkv, "bqkv": bqkv, "bv": bv,
        "wproj": wproj, "bproj": S * proj_b,
        "w1h": w1h, "w1l": w1l, "b1": b1,
        "w2h": w2h, "w2l": w2l, "b2": S * fc2_b,
    }
    in_maps = []
    for c in range(8):
        m = dict(shared)
        m["x"] = np.ascontiguousarray(
            S * x[c * B_PER_CORE:(c + 1) * B_PER_CORE].reshape(T, C))
        in_maps.append(m)
    return in_maps


def kernel(**inputs):
    global _CACHED_NC
    b1_host = (np.asarray(inputs["fc1_b"], np.float32)
               + np.asarray(inputs["fc1_w"], np.float32)
               @ np.asarray(inputs["ln2_b"], np.float32))
    b1_zero = bool(np.all(b1_host == 0.0))
    bqkv_host = (np.asarray(inputs["qkv_w"], np.float32)
                 @ np.asarray(inputs["ln1_b"], np.float32))
    bv_zero = bool(np.all(bqkv_host[2 * C:] == 0.0))
    bqk_zero = bool(np.all(bqkv_host[:2 * C] == 0.0))
    bproj_zero = bool(np.all(np.asarray(inputs["proj_b"]) == 0.0))
    b2_zero = bool(np.all(np.asarray(inputs["fc2_b"]) == 0.0))
    key = (b1_zero, bv_zero, bproj_zero, b2_zero, bqk_zero)
    if _CACHED_NC is None or getattr(_CACHED_NC, "_spec", None) != key:
        _CACHED_NC = _build(b1_zero=b1_zero, bv_zero=bv_zero,
                            bproj_zero=bproj_zero, b2_zero=b2_zero,
                            bqk_zero=bqk_zero)
        _CACHED_NC._spec = key
    nc = _CACHED_NC
    in_maps = _prep_host(inputs)
    trace = os.environ.get("TRN_KERNEL_TRACE", "0") == "1"
    res = run_bass_kernel_spmd(nc, in_maps, core_ids=list(range(8)),
                               trace=trace)
    if trace and res.exec_time_ns is not None:
        print(f"HW exec time: {res.exec_time_ns} ns")
        print(f"mean exec time: {res.mean_exec_time_ns} ns")
        if res.instructions_and_trace is not None:
            print(f"trace: {res.instructions_and_trace[1]}")
    out = np.stack([
        res.results[c]["out"].reshape(B_PER_CORE, SEQ, C) for c in range(8)
    ]).reshape(16, SEQ, C)
    return (out / S).astype(np.float32)


# revision 33
# speedup vs baseline: 1.0087x; 1.0087x over previous
"""Trainium2 Bass kernel for a ViT-style transformer block (nn_Block_11132555231612).

Data-parallel over batch across 8 NeuronCores (2 sequences of 1024 tokens per
core). fp8e4m3 DoubleRow matmuls (0.5 cyc/row, 256-deep contraction) carry
QKV / AV / proj / fc1 / fc2; scores stay bf16 (64-deep contraction). The
residual stream is held at 32x scale (x scaled on host, weights scaled to
match, output unscaled on host) so every fp8 weight scale folds away without
device fixups; softmax normalization keeps the appended-ones-column trick
(column value = 32 cancels the V scale). fc1/fc2 weights use hi+lo double-fp8
planes accumulated in PSUM to recover near-bf16 weight precision. LN applies
run on the (otherwise idle) GPSIMD engine to keep the scalar engine free for
softmax exp, which is the critical path of the attention phase.
"""

import os
import sys

sys.path.insert(0, "/opt/trn_rl_repo")

import numpy as np
import ml_dtypes

import concourse.bass as bass
import concourse.mybir as mybir
import concourse.tile as tile
from concourse import bacc
from concourse.bass_utils import run_bass_kernel_spmd
from concourse.masks import make_identity
from contextlib import ExitStack

F32 = mybir.dt.float32
BF16 = mybir.dt.bfloat16
FP8 = mybir.dt.float8e4
NP8 = ml_dtypes.float8_e4m3
AF = mybir.ActivationFunctionType
DR = mybir.MatmulPerfMode.DoubleRow
ALU = mybir.AluOpType

P = 128
B_PER_CORE = 2
SEQ = 1024
T = B_PER_CORE * SEQ          # 2048 tokens per core
C = 768
H = 12
HD = 64
HID = 3072
KS = C // P                   # 6
HS = HID // P                 # 24
NT = T // P                   # 16 token tiles
EPS = 1e-5
SCALE = HD ** -0.5            # 0.125
S = 32.0                      # residual / weight scale
VP = 80                       # padded V row (65 used): 16B dual-fp8 ldweights

_CACHED_NC = None


class TileKernel:
    b1_zero = False
    bv_zero = False
    bproj_zero = False
    b2_zero = False
    bqk_zero = False
    w1x2 = True
    w2x2 = True

    def __init__(self, nc):
        self.nc = nc
        self.stack = ExitStack()
        self.tc = None

    def __enter__(self):
        self.tc = self.stack.enter_context(tile.TileContext(self.nc))
        return self

    def __exit__(self, *exc):
        return self.stack.__exit__(*exc)

    def ln_tile(self, xt, dst, dst_col, work, psum_ln, eps_t, ident):
        """LN of one token-major tile xt [P, C] -> feature-major dst tile
        columns [P, KS, P] at dst[:, :, dst_col:dst_col+P].

        Stats on DVE, apply on GPSIMD (tensor_scalar), transpose on PE in
        bf16 (hw rejects fp8 transposes), psum->sbuf copy converts dtype.
        """
        nc = self.nc
        st = work.tile([P, 2, 6], F32, tag="bnstats")
        xg = xt.rearrange("p (s d) -> p s d", s=2)
        for s in range(2):
            nc.vector.bn_stats(st[:, s, :], xg[:, s, :])
        mv = work.tile([P, 2], F32, tag="mv")
        nc.vector.bn_aggr(mv[:], st[:])
        sdv = work.tile([P, 1], F32, tag="sdv")
        nc.scalar.activation(sdv[:], mv[:, 1:2], AF.Sqrt, bias=eps_t[:])
        rstd = work.tile([P, 1], F32, tag="rstd")
        nc.vector.reciprocal(rstd[:], sdv[:])
        nmu = work.tile([P, 1], F32, tag="nmu")
        nc.vector.tensor_scalar_mul(nmu[:], mv[:, 0:1], -1.0)
        xn = work.tile([P, C], BF16, tag="xn")
        nc.gpsimd.tensor_scalar(xn[:], xt, nmu[:], rstd[:],
                                op0=ALU.add, op1=ALU.mult)
        pt = psum_ln.tile([P, KS, P], BF16, tag="tp")
        for c in range(KS):
            nc.tensor.transpose(pt[:, c, :], xn[:, c * P:(c + 1) * P],
                                ident[:])
        nc.any.tensor_copy(dst[:, :, dst_col:dst_col + P], pt[:])

    def run(self, x_d, out_d, wqkv_d, bqkv_d, bv_d, wproj_d, bproj_d,
            w1h_d, w1l_d, b1_d, w2h_d, w2l_d, b2_d):
        nc, tc, S_ = self.nc, self.tc, self.stack
        const = S_.enter_context(tc.tile_pool(name="const", bufs=1))
        xpool = S_.enter_context(tc.tile_pool(name="xres", bufs=1))
        work = S_.enter_context(tc.tile_pool(name="work", bufs=3))

        ident16 = const.tile([P, P], BF16)
        make_identity(nc, ident16[:])
        eps_t = const.tile([P, 1], F32)
        nc.vector.memset(eps_t[:], EPS * S * S)
        if not self.bqk_zero:
            bqkv_sb = const.tile([P, 12], F32)
            nc.sync.dma_start(bqkv_sb[:], bqkv_d[:])
        if not self.b1_zero:
            b1_sb = const.tile([P, HS], F32)
            nc.sync.dma_start(b1_sb[:], b1_d[:])
        x_sb = xpool.tile([P, NT, C], F32)
        xr = x_d[:].rearrange("(n p) c -> p n c", p=P)
        for t4 in range(8):
            nc.sync.dma_start(x_sb[:, t4 * 2:(t4 + 1) * 2, :],
                              xr[:, t4 * 2:(t4 + 1) * 2, :])

        ablate = os.environ.get("TRN_ABLATE", "")
        # oT / wproj live until proj; everything else attention-local frees
        # before the MLP weights arrive.
        o_p = S_.enter_context(tc.tile_pool(name="oT", bufs=1))
        wp_p = S_.enter_context(tc.tile_pool(name="wpp", bufs=1))
        oT = o_p.tile([P, KS, T], FP8)
        if ablate != "skip_attn":
         with ExitStack() as attn_win:
            wproj_sb = wp_p.tile([P, KS, C], FP8)
            nc.sync.dma_start(wproj_sb[:], wproj_d[:])
            qkT_p = attn_win.enter_context(tc.tile_pool(name="qkT", bufs=1))
            v_p = attn_win.enter_context(tc.tile_pool(name="vtile", bufs=1))
            wq_p = attn_win.enter_context(tc.tile_pool(name="wqp", bufs=1))
            qkT = qkT_p.tile([P, 12, T], BF16)
            V_sb = v_p.tile([P, NT, H, VP], FP8)
            wqkv_sb = wq_p.tile([P, KS, 3 * C], FP8)
            for c2 in range(3):
                nc.sync.dma_start(wqkv_sb[:, c2 * 2:c2 * 2 + 2, :],
                                  wqkv_d[:, c2 * 2:c2 * 2 + 2, :])

            with ExitStack() as s1:
                xnT_p = s1.enter_context(tc.tile_pool(name="xnT1", bufs=1))
                psum_ln = s1.enter_context(
                    tc.tile_pool(name="psln", bufs=2, space="PSUM"))
                psum_mm = s1.enter_context(
                    tc.tile_pool(name="psmm", bufs=2, space="PSUM"))
                bv_p = s1.enter_context(tc.tile_pool(name="bvp", bufs=1))

                if not self.bv_zero:
                    bv_bc = bv_p.tile([P, C], F32)
                    nc.sync.dma_start(bv_bc[:],
                                      bv_d[:].partition_broadcast(P))

                xnT = xnT_p.tile([P, KS, T], FP8)
                for t in range(NT):
                    self.ln_tile(x_sb[:, t, :], xnT, t * P, work, psum_ln,
                                 eps_t, ident16)

                # V token-major with S-valued column at slot 64 (denominator
                # trick: cancels the S scale of V on normalization)
                nc.vector.memset(V_sb[:, :, :, HD], S)
                for t in range(NT):
                    psv = psum_mm.tile([P, C], F32, tag="psv")
                    for (n0, nsz) in ((0, 512), (512, 256)):
                        for c2 in range(3):
                            nc.tensor.matmul(
                                psv[:, n0:n0 + nsz],
                                xnT[:, c2 * 2:c2 * 2 + 2, t * P:(t + 1) * P],
                                wqkv_sb[:, c2 * 2:c2 * 2 + 2,
                                        2 * C + n0:2 * C + n0 + nsz],
                                start=(c2 == 0), stop=(c2 == 2),
                                perf_mode=DR)
                    if self.bv_zero:
                        nc.any.tensor_copy(
                            V_sb[:, t, :, 0:HD],
                            psv[:].rearrange("p (h d) -> p h d", h=H))
                    else:
                        nc.vector.tensor_add(
                            V_sb[:, t, :, 0:HD],
                            psv[:].rearrange("p (h d) -> p h d", h=H),
                            bv_bc[:].rearrange("p (h d) -> p h d", h=H))

                # q^T / k^T feature-major, head-pair order (q then k per pair)
                for oct in [x for p_ in range(6) for x in (p_, 6 + p_)]:
                    for nch in range(T // 512):
                        ps = psum_mm.tile([P, 512], F32, tag="ps")
                        for c2 in range(3):
                            nc.tensor.matmul(
                                ps[:],
                                wqkv_sb[:, c2 * 2:c2 * 2 + 2,
                                        oct * P:(oct + 1) * P],
                                xnT[:, c2 * 2:c2 * 2 + 2,
                                    nch * 512:(nch + 1) * 512],
                                start=(c2 == 0), stop=(c2 == 2),
                                perf_mode=DR)
                        if self.bqk_zero:
                            nc.any.tensor_copy(
                                qkT[:, oct, nch * 512:(nch + 1) * 512], ps[:])
                        else:
                            nc.vector.tensor_scalar_add(
                                qkT[:, oct, nch * 512:(nch + 1) * 512], ps[:],
                                bqkv_sb[:, oct:oct + 1])

            # ---- attention ----
            with ExitStack() as s2:
                psum_s = s2.enter_context(
                    tc.tile_pool(name="pss", bufs=3, space="PSUM"))
                psum_o = s2.enter_context(
                    tc.tile_pool(name="pso", bufs=2, space="PSUM"))
                awork = s2.enter_context(tc.tile_pool(name="awork", bufs=3))
                for b in range(B_PER_CORE):
                    for h in range(H):
                        po = (h % 2) * 64
                        oq, ok = h // 2, 6 + h // 2
                        for qc in range(SEQ // 512):
                            qs = b * SEQ + qc * 512
                            pso = psum_o.tile([P, 512], F32, tag="pso")
                            for kt2 in range(SEQ // (2 * P)):
                                pss = psum_s.tile([P, 2, 512], F32, tag="pss")
                                for j in range(2):
                                    ko = b * SEQ + (2 * kt2 + j) * P
                                    nc.tensor.matmul(
                                        pss[:, j, :],
                                        qkT[po:po + HD, ok, ko:ko + P],
                                        qkT[po:po + HD, oq, qs:qs + 512],
                                        start=True, stop=True)
                                pr = awork.tile([P, 2, 512], FP8, tag="probs")
                                nc.scalar.activation(pr[:], pss[:], AF.Exp,
                                                     scale=SCALE / (S * S))
                                nc.tensor.matmul(
                                    pso[0:HD + 1, :],
                                    V_sb[:, b * 8 + 2 * kt2:b * 8 + 2 * kt2 + 2,
                                         h, 0:HD + 1],
                                    pr[:],
                                    start=(kt2 == 0),
                                    stop=(kt2 == SEQ // (2 * P) - 1),
                                    perf_mode=DR)
                            rc = awork.tile([P, 512], F32, tag="recip")
                            nc.vector.reciprocal(rc[HD:HD + 1, :],
                                                 pso[HD:HD + 1, :])
                            rc0 = awork.tile([1, 512], F32, tag="rc0")
                            nc.sync.dma_start(rc0[:], rc[HD:HD + 1, :])
                            rbc = awork.tile([HD, 512], F32, tag="rbc")
                            nc.gpsimd.partition_broadcast(
                                rbc[:], rc0[0:1, :], channels=HD)
                            if h % 2 == 0:
                                nc.vector.tensor_mul(
                                    oT[0:HD, h // 2, qs:qs + 512],
                                    pso[0:HD, :], rbc[:])
                            else:
                                osc = awork.tile([HD, 512], FP8, tag="osc")
                                nc.vector.tensor_mul(osc[:], pso[0:HD, :],
                                                     rbc[:])
                                nc.sync.dma_start(
                                    oT[64:128, h // 2, qs:qs + 512], osc[:])

        # ---- MLP weights (DMA overlaps proj/LN2), proj + LN2, MLP ----
        if ablate != "skip_mlp":
         with ExitStack() as s4:
            w_p = s4.enter_context(tc.tile_pool(name="wmlp", bufs=1))
            xnT_p2 = s4.enter_context(tc.tile_pool(name="xnT2", bufs=1))
            h_p = s4.enter_context(tc.tile_pool(name="hT", bufs=2))
            b2_p = s4.enter_context(tc.tile_pool(name="b2p", bufs=1))

            w1_planes = []
            w1h_sb = w_p.tile([P, KS, HID], FP8)
            for q in range(8):
                nc.sync.dma_start(w1h_sb[:, :, q * 384:(q + 1) * 384],
                                  w1h_d[:, :, q * 384:(q + 1) * 384])
            w1_planes.append(w1h_sb)
            if self.w1x2:
                w1l_sb = w_p.tile([P, KS, HID], FP8)
                for q in range(8):
                    nc.sync.dma_start(w1l_sb[:, :, q * 384:(q + 1) * 384],
                                      w1l_d[:, :, q * 384:(q + 1) * 384])
                w1_planes.append(w1l_sb)
            w2_planes = []
            w2h_sb = w_p.tile([P, HS, C], FP8)
            for c3 in range(4):
                nc.sync.dma_start(w2h_sb[:, c3 * 6:c3 * 6 + 6, :],
                                  w2h_d[:, c3 * 6:c3 * 6 + 6, :])
            w2_planes.append(w2h_sb)
            if self.w2x2:
                w2l_sb = w_p.tile([P, HS, C], FP8)
                for c3 in range(4):
                    nc.sync.dma_start(w2l_sb[:, c3 * 6:c3 * 6 + 6, :],
                                      w2l_d[:, c3 * 6:c3 * 6 + 6, :])
                w2_planes.append(w2l_sb)

            if not self.b2_zero:
                b2_bc = b2_p.tile([P, C], F32)
                nc.sync.dma_start(b2_bc[:], b2_d[:].partition_broadcast(P))

            xnT2 = xnT_p2.tile([P, KS, T], FP8)
            # proj + residual + LN2, interleaved per token tile
            with ExitStack() as s3:
                psum_p = s3.enter_context(
                    tc.tile_pool(name="psp", bufs=2, space="PSUM"))
                psum_ln2 = s3.enter_context(
                    tc.tile_pool(name="psln2", bufs=2, space="PSUM"))
                bp_p = s3.enter_context(tc.tile_pool(name="bpp", bufs=1))
                if not self.bproj_zero:
                    bproj_bc = bp_p.tile([P, C], F32)
                    nc.sync.dma_start(bproj_bc[:],
                                      bproj_d[:].partition_broadcast(P))
                for t in range(NT):
                    if ablate != "skip_attn":
                        psp = psum_p.tile([P, C], F32, tag="psp")
                        for (n0, nsz) in ((0, 512), (512, 256)):
                            for c2 in range(3):
                                nc.tensor.matmul(
                                    psp[:, n0:n0 + nsz],
                                    oT[:, c2 * 2:c2 * 2 + 2, t * P:(t + 1) * P],
                                    wproj_sb[:, c2 * 2:c2 * 2 + 2, n0:n0 + nsz],
                                    start=(c2 == 0), stop=(c2 == 2),
                                    perf_mode=DR)
                        nc.vector.tensor_add(x_sb[:, t, :], x_sb[:, t, :],
                                             psp[:])
                        if not self.bproj_zero:
                            nc.vector.tensor_add(x_sb[:, t, :], x_sb[:, t, :],
                                                 bproj_bc[:])
                    self.ln_tile(x_sb[:, t, :], xnT2, t * P, work, psum_ln2,
                                 eps_t, ident16)

            psum_1 = s4.enter_context(
                tc.tile_pool(name="ps1", bufs=2, space="PSUM"))
            psum_2 = s4.enter_context(
                tc.tile_pool(name="ps2", bufs=2, space="PSUM"))

            n1 = 3 * len(w1_planes)
            n2 = 12 * len(w2_planes)
            for tq in range(T // 512):
                t0 = tq * 512
                hT = h_p.tile([P, HS, 512], FP8, tag="hT")
                for hp in range(12):               # hidden-feature pairs
                    ps1 = psum_1.tile([P, 2, 512], F32, tag="ps1")
                    for j in range(2):
                        i = 0
                        for w1p in w1_planes:
                            for c3 in range(3):
                                nc.tensor.matmul(
                                    ps1[:, j, :],
                                    w1p[:, c3 * 2:c3 * 2 + 2,
                                        (hp * 2 + j) * P:(hp * 2 + j + 1) * P],
                                    xnT2[:, c3 * 2:c3 * 2 + 2, t0:t0 + 512],
                                    start=(i == 0), stop=(i == n1 - 1),
                                    perf_mode=DR)
                                i += 1
                    if self.b1_zero:
                        nc.scalar.activation(
                            hT[:, hp * 2:hp * 2 + 2, :].rearrange(
                                "p a b -> p (a b)"),
                            ps1[:].rearrange("p a b -> p (a b)"),
                            AF.Gelu, scale=1.0 / S)
                    else:
                        for j in range(2):
                            nc.scalar.activation(
                                hT[:, hp * 2 + j, :], ps1[:, j, :],
                                AF.Gelu,
                                bias=b1_sb[:, hp * 2 + j:hp * 2 + j + 1],
                                scale=1.0 / S)
                for tt in range(4):
                    tg = tq * 4 + tt
                    ps2 = psum_2.tile([P, C], F32, tag="ps2")
                    for (n0, nsz) in ((0, 512), (512, 256)):
                        i = 0
                        for hp in range(12):
                            for w2p in w2_planes:
                                nc.tensor.matmul(
                                    ps2[:, n0:n0 + nsz],
                                    hT[:, hp * 2:hp * 2 + 2,
                                       tt * P:(tt + 1) * P],
                                    w2p[:, hp * 2:hp * 2 + 2, n0:n0 + nsz],
                                    start=(i == 0), stop=(i == n2 - 1),
                                    perf_mode=DR)
                                i += 1
                    nc.vector.tensor_add(x_sb[:, tg, :], x_sb[:, tg, :],
                                         ps2[:])
                    if not self.b2_zero:
                        nc.vector.tensor_add(x_sb[:, tg, :], x_sb[:, tg, :],
                                             b2_bc[:])
                nc.sync.dma_start(
                    out_d[:].rearrange("(n p) c -> p n c", p=P)[:, tq * 4:tq * 4 + 4, :],
                    x_sb[:, tq * 4:tq * 4 + 4, :])


def _build(b1_zero=False, bv_zero=False, bproj_zero=False, b2_zero=False,
           bqk_zero=False, w1x2=True, w2x2=True):
    nc = bacc.Bacc(None, target_bir_lowering=False, debug=False)

    x_d = nc.dram_tensor("x", [T, C], F32, kind="ExternalInput")
    out_d = nc.dram_tensor("out", [T, C], F32, kind="ExternalOutput")
    wqkv_d = nc.dram_tensor("wqkv", [P, KS, 3 * C], FP8, kind="ExternalInput")
    bqkv_d = nc.dram_tensor("bqkv", [P, 12], F32, kind="ExternalInput")
    bv_d = nc.dram_tensor("bv", [C], F32, kind="ExternalInput")
    wproj_d = nc.dram_tensor("wproj", [P, KS, C], FP8, kind="ExternalInput")
    bproj_d = nc.dram_tensor("bproj", [C], F32, kind="ExternalInput")
    w1h_d = nc.dram_tensor("w1h", [P, KS, HID], FP8, kind="ExternalInput")
    w1l_d = nc.dram_tensor("w1l", [P, KS, HID], FP8, kind="ExternalInput")
    b1_d = nc.dram_tensor("b1", [P, HS], F32, kind="ExternalInput")
    w2h_d = nc.dram_tensor("w2h", [P, HS, C], FP8, kind="ExternalInput")
    w2l_d = nc.dram_tensor("w2l", [P, HS, C], FP8, kind="ExternalInput")
    b2_d = nc.dram_tensor("b2", [C], F32, kind="ExternalInput")
    with TileKernel(nc) as tk:
        tk.b1_zero = b1_zero
        tk.bqk_zero = bqk_zero
        tk.bv_zero = bv_zero
        tk.bproj_zero = bproj_zero
        tk.b2_zero = b2_zero
        tk.w1x2 = w1x2
        tk.w2x2 = w2x2
        tk.run(x_d, out_d, wqkv_d, bqkv_d, bv_d, wproj_d, bproj_d,
               w1h_d, w1l_d, b1_d, w2h_d, w2l_d, b2_d)

    nc.compile()
    return nc


def _hilo(w):
    hi = w.astype(NP8)
    lo = (w - hi.astype(np.float32)).astype(NP8)
    return hi, lo


def _prep_host(inputs):
    f = lambda a: np.asarray(a, dtype=np.float32)
    x = f(inputs["x"])
    ln1_g, ln1_b = f(inputs["ln1_g"]), f(inputs["ln1_b"])
    ln2_g, ln2_b = f(inputs["ln2_g"]), f(inputs["ln2_b"])
    qkv_w = f(inputs["qkv_w"])
    proj_w, proj_b = f(inputs["proj_w"]), f(inputs["proj_b"])
    fc1_w, fc1_b = f(inputs["fc1_w"]), f(inputs["fc1_b"])
    fc2_w, fc2_b = f(inputs["fc2_w"]), f(inputs["fc2_b"])

    wqkv = np.ascontiguousarray(
        (qkv_w * ln1_g[None, :] * S).T.reshape(KS, P, 3 * C).transpose(1, 0, 2)
    ).astype(NP8)
    bqkv_full = S * (qkv_w @ ln1_b)                # [2304], S-scaled
    bqkv = np.ascontiguousarray(bqkv_full[:2 * C].reshape(12, P).T)
    bv = np.ascontiguousarray(bqkv_full[2 * C:])
    wproj = np.ascontiguousarray(
        (proj_w * S).T.reshape(KS, P, C).transpose(1, 0, 2)).astype(NP8)
    w1 = np.ascontiguousarray(
        (fc1_w * ln2_g[None, :] * S).T.reshape(KS, P, HID).transpose(1, 0, 2))
    w1h, w1l = _hilo(w1)
    b1 = np.ascontiguousarray((fc1_b + fc1_w @ ln2_b).reshape(HS, P).T)
    w2 = np.ascontiguousarray(
        (fc2_w * S).T.reshape(HS, P, C).transpose(1, 0, 2))
    w2h, w2l = _hilo(w2)

    shared = {
        "wqkv": wqkv, "bqkv": bqkv, "bv": bv,
        "wproj": wproj, "bproj": S * proj_b,
        "w1h": w1h, "w1l": w1l, "b1": b1,
        "w2h": w2h, "w2l": w2l, "b2": S * fc2_b,
    }
    in_maps = []
    for c in range(8):
        m = dict(shared)
        m["x"] = np.ascontiguousarray(
            S * x[c * B_PER_CORE:(c + 1) * B_PER_CORE].reshape(T, C))
        in_maps.append(m)
    return in_maps


def kernel(**inputs):
    global _CACHED_NC
    b1_host = (np.asarray(inputs["fc1_b"], np.float32)
               + np.asarray(inputs["fc1_w"], np.float32)
               @ np.asarray(inputs["ln2_b"], np.float32))
    b1_zero = bool(np.all(b1_host == 0.0))
    bqkv_host = (np.asarray(inputs["qkv_w"], np.float32)
                 @ np.asarray(inputs["ln1_b"], np.float32))
    bv_zero = bool(np.all(bqkv_host[2 * C:] == 0.0))
    bqk_zero = bool(np.all(bqkv_host[:2 * C] == 0.0))
    bproj_zero = bool(np.all(np.asarray(inputs["proj_b"]) == 0.0))
    b2_zero = bool(np.all(np.asarray(inputs["fc2_b"]) == 0.0))
    key = (b1_zero, bv_zero, bproj_zero, b2_zero, bqk_zero)
    if _CACHED_NC is None or getattr(_CACHED_NC, "_spec", None) != key:
        _CACHED_NC = _build(b1_zero=b1_zero, bv_zero=bv_zero,
                            bproj_zero=bproj_zero, b2_zero=b2_zero,
                            bqk_zero=bqk_zero)
        _CACHED_NC._spec = key
    nc = _CACHED_NC
    in_maps = _prep_host(inputs)
    trace = os.environ.get("TRN_KERNEL_TRACE", "0") == "1"
    res = run_bass_kernel_spmd(nc, in_maps, core_ids=list(range(8)),
                               trace=trace)
    if trace and res.exec_time_ns is not None:
        print(f"HW exec time: {res.exec_time_ns} ns")
        print(f"mean exec time: {res.mean_exec_time_ns} ns")
        if res.instructions_and_trace is not None:
            print(f"trace: {res.instructions_and_trace[1]}")
    out = np.stack([
        res.results[c]["out"].reshape(B_PER_CORE, SEQ, C) for c in range(8)
    ]).reshape(16, SEQ, C)
    return (out / S).astype(np.float32)


# revision 34
# speedup vs baseline: 1.0088x; 1.0001x over previous
"""Trainium2 Bass kernel for a ViT-style transformer block (nn_Block_11132555231612).

Data-parallel over batch across 8 NeuronCores (2 sequences of 1024 tokens per
core). fp8e4m3 DoubleRow matmuls (0.5 cyc/row, 256-deep contraction) carry
QKV / AV / proj / fc1 / fc2; scores stay bf16 (64-deep contraction). The
residual stream is held at 32x scale (x scaled on host, weights scaled to
match, output unscaled on host) so every fp8 weight scale folds away without
device fixups; softmax normalization keeps the appended-ones-column trick
(column value = 32 cancels the V scale). fc1/fc2 weights use hi+lo double-fp8
planes accumulated in PSUM to recover near-bf16 weight precision. LN applies
run on the (otherwise idle) GPSIMD engine to keep the scalar engine free for
softmax exp, which is the critical path of the attention phase.
"""

import os
import sys

sys.path.insert(0, "/opt/trn_rl_repo")

import numpy as np
import ml_dtypes

import concourse.bass as bass
import concourse.mybir as mybir
import concourse.tile as tile
from concourse import bacc
from concourse.bass_utils import run_bass_kernel_spmd
from concourse.masks import make_identity
from contextlib import ExitStack

F32 = mybir.dt.float32
BF16 = mybir.dt.bfloat16
FP8 = mybir.dt.float8e4
NP8 = ml_dtypes.float8_e4m3
AF = mybir.ActivationFunctionType
DR = mybir.MatmulPerfMode.DoubleRow
ALU = mybir.AluOpType

P = 128
B_PER_CORE = 2
SEQ = 1024
T = B_PER_CORE * SEQ          # 2048 tokens per core
C = 768
H = 12
HD = 64
HID = 3072
KS = C // P                   # 6
HS = HID // P                 # 24
NT = T // P                   # 16 token tiles
EPS = 1e-5
SCALE = HD ** -0.5            # 0.125
S = 32.0                      # residual / weight scale
VP = 80                       # padded V row (65 used): 16B dual-fp8 ldweights

_CACHED_NC = None


class TileKernel:
    b1_zero = False
    bv_zero = False
    bproj_zero = False
    b2_zero = False
    bqk_zero = False
    w1x2 = True
    w2x2 = True

    def __init__(self, nc):
        self.nc = nc
        self.stack = ExitStack()
        self.tc = None

    def __enter__(self):
        self.tc = self.stack.enter_context(tile.TileContext(self.nc))
        return self

    def __exit__(self, *exc):
        return self.stack.__exit__(*exc)

    def ln_tile(self, xt, dst, dst_col, work, psum_ln, eps_t, ident):
        """LN of one token-major tile xt [P, C] -> feature-major dst tile
        columns [P, KS, P] at dst[:, :, dst_col:dst_col+P].

        Stats on DVE, apply on GPSIMD (tensor_scalar), transpose on PE in
        bf16 (hw rejects fp8 transposes), psum->sbuf copy converts dtype.
        """
        nc = self.nc
        st = work.tile([P, 2, 6], F32, tag="bnstats")
        xg = xt.rearrange("p (s d) -> p s d", s=2)
        for s in range(2):
            nc.vector.bn_stats(st[:, s, :], xg[:, s, :])
        mv = work.tile([P, 2], F32, tag="mv")
        nc.vector.bn_aggr(mv[:], st[:])
        sdv = work.tile([P, 1], F32, tag="sdv")
        nc.scalar.activation(sdv[:], mv[:, 1:2], AF.Sqrt, bias=eps_t[:])
        rstd = work.tile([P, 1], F32, tag="rstd")
        nc.vector.reciprocal(rstd[:], sdv[:])
        nmu = work.tile([P, 1], F32, tag="nmu")
        nc.vector.tensor_scalar_mul(nmu[:], mv[:, 0:1], -1.0)
        xn = work.tile([P, C], BF16, tag="xn")
        nc.gpsimd.tensor_scalar(xn[:], xt, nmu[:], rstd[:],
                                op0=ALU.add, op1=ALU.mult)
        pt = psum_ln.tile([P, KS, P], BF16, tag="tp")
        for c in range(KS):
            nc.tensor.transpose(pt[:, c, :], xn[:, c * P:(c + 1) * P],
                                ident[:])
        nc.any.tensor_copy(dst[:, :, dst_col:dst_col + P], pt[:])

    def run(self, x_d, out_d, wqkv_d, bqkv_d, bv_d, wproj_d, bproj_d,
            w1h_d, w1l_d, b1_d, w2h_d, w2l_d, b2_d):
        nc, tc, S_ = self.nc, self.tc, self.stack
        const = S_.enter_context(tc.tile_pool(name="const", bufs=1))
        xpool = S_.enter_context(tc.tile_pool(name="xres", bufs=1))
        work = S_.enter_context(tc.tile_pool(name="work", bufs=5))

        ident16 = const.tile([P, P], BF16)
        make_identity(nc, ident16[:])
        eps_t = const.tile([P, 1], F32)
        nc.vector.memset(eps_t[:], EPS * S * S)
        if not self.bqk_zero:
            bqkv_sb = const.tile([P, 12], F32)
            nc.sync.dma_start(bqkv_sb[:], bqkv_d[:])
        if not self.b1_zero:
            b1_sb = const.tile([P, HS], F32)
            nc.sync.dma_start(b1_sb[:], b1_d[:])
        x_sb = xpool.tile([P, NT, C], F32)
        xr = x_d[:].rearrange("(n p) c -> p n c", p=P)
        for t4 in range(8):
            nc.sync.dma_start(x_sb[:, t4 * 2:(t4 + 1) * 2, :],
                              xr[:, t4 * 2:(t4 + 1) * 2, :])

        ablate = os.environ.get("TRN_ABLATE", "")
        # oT / wproj live until proj; everything else attention-local frees
        # before the MLP weights arrive.
        o_p = S_.enter_context(tc.tile_pool(name="oT", bufs=1))
        wp_p = S_.enter_context(tc.tile_pool(name="wpp", bufs=1))
        oT = o_p.tile([P, KS, T], FP8)
        if ablate != "skip_attn":
         with ExitStack() as attn_win:
            wproj_sb = wp_p.tile([P, KS, C], FP8)
            nc.sync.dma_start(wproj_sb[:], wproj_d[:])
            qkT_p = attn_win.enter_context(tc.tile_pool(name="qkT", bufs=1))
            v_p = attn_win.enter_context(tc.tile_pool(name="vtile", bufs=1))
            wq_p = attn_win.enter_context(tc.tile_pool(name="wqp", bufs=1))
            qkT = qkT_p.tile([P, 12, T], BF16)
            V_sb = v_p.tile([P, NT, H, VP], FP8)
            wqkv_sb = wq_p.tile([P, KS, 3 * C], FP8)
            for c2 in range(3):
                nc.sync.dma_start(wqkv_sb[:, c2 * 2:c2 * 2 + 2, :],
                                  wqkv_d[:, c2 * 2:c2 * 2 + 2, :])

            with ExitStack() as s1:
                xnT_p = s1.enter_context(tc.tile_pool(name="xnT1", bufs=1))
                psum_ln = s1.enter_context(
                    tc.tile_pool(name="psln", bufs=2, space="PSUM"))
                psum_mm = s1.enter_context(
                    tc.tile_pool(name="psmm", bufs=2, space="PSUM"))
                bv_p = s1.enter_context(tc.tile_pool(name="bvp", bufs=1))

                if not self.bv_zero:
                    bv_bc = bv_p.tile([P, C], F32)
                    nc.sync.dma_start(bv_bc[:],
                                      bv_d[:].partition_broadcast(P))

                xnT = xnT_p.tile([P, KS, T], FP8)
                for t in range(NT):
                    self.ln_tile(x_sb[:, t, :], xnT, t * P, work, psum_ln,
                                 eps_t, ident16)

                # V token-major with S-valued column at slot 64 (denominator
                # trick: cancels the S scale of V on normalization)
                nc.vector.memset(V_sb[:, :, :, HD], S)
                for t in range(NT):
                    psv = psum_mm.tile([P, C], F32, tag="psv")
                    for (n0, nsz) in ((0, 512), (512, 256)):
                        for c2 in range(3):
                            nc.tensor.matmul(
                                psv[:, n0:n0 + nsz],
                                xnT[:, c2 * 2:c2 * 2 + 2, t * P:(t + 1) * P],
                                wqkv_sb[:, c2 * 2:c2 * 2 + 2,
                                        2 * C + n0:2 * C + n0 + nsz],
                                start=(c2 == 0), stop=(c2 == 2),
                                perf_mode=DR)
                    if self.bv_zero:
                        nc.any.tensor_copy(
                            V_sb[:, t, :, 0:HD],
                            psv[:].rearrange("p (h d) -> p h d", h=H))
                    else:
                        nc.vector.tensor_add(
                            V_sb[:, t, :, 0:HD],
                            psv[:].rearrange("p (h d) -> p h d", h=H),
                            bv_bc[:].rearrange("p (h d) -> p h d", h=H))

                # q^T / k^T feature-major, head-pair order (q then k per pair)
                for oct in [x for p_ in range(6) for x in (p_, 6 + p_)]:
                    for nch in range(T // 512):
                        ps = psum_mm.tile([P, 512], F32, tag="ps")
                        for c2 in range(3):
                            nc.tensor.matmul(
                                ps[:],
                                wqkv_sb[:, c2 * 2:c2 * 2 + 2,
                                        oct * P:(oct + 1) * P],
                                xnT[:, c2 * 2:c2 * 2 + 2,
                                    nch * 512:(nch + 1) * 512],
                                start=(c2 == 0), stop=(c2 == 2),
                                perf_mode=DR)
                        if self.bqk_zero:
                            nc.any.tensor_copy(
                                qkT[:, oct, nch * 512:(nch + 1) * 512], ps[:])
                        else:
                            nc.vector.tensor_scalar_add(
                                qkT[:, oct, nch * 512:(nch + 1) * 512], ps[:],
                                bqkv_sb[:, oct:oct + 1])

            # ---- attention ----
            with ExitStack() as s2:
                psum_s = s2.enter_context(
                    tc.tile_pool(name="pss", bufs=3, space="PSUM"))
                psum_o = s2.enter_context(
                    tc.tile_pool(name="pso", bufs=2, space="PSUM"))
                awork = s2.enter_context(tc.tile_pool(name="awork", bufs=3))
                for b in range(B_PER_CORE):
                    for h in range(H):
                        po = (h % 2) * 64
                        oq, ok = h // 2, 6 + h // 2
                        for qc in range(SEQ // 512):
                            qs = b * SEQ + qc * 512
                            pso = psum_o.tile([P, 512], F32, tag="pso")
                            for kt2 in range(SEQ // (2 * P)):
                                pss = psum_s.tile([P, 2, 512], F32, tag="pss")
                                for j in range(2):
                                    ko = b * SEQ + (2 * kt2 + j) * P
                                    nc.tensor.matmul(
                                        pss[:, j, :],
                                        qkT[po:po + HD, ok, ko:ko + P],
                                        qkT[po:po + HD, oq, qs:qs + 512],
                                        start=True, stop=True)
                                pr = awork.tile([P, 2, 512], FP8, tag="probs")
                                nc.scalar.activation(pr[:], pss[:], AF.Exp,
                                                     scale=SCALE / (S * S))
                                nc.tensor.matmul(
                                    pso[0:HD + 1, :],
                                    V_sb[:, b * 8 + 2 * kt2:b * 8 + 2 * kt2 + 2,
                                         h, 0:HD + 1],
                                    pr[:],
                                    start=(kt2 == 0),
                                    stop=(kt2 == SEQ // (2 * P) - 1),
                                    perf_mode=DR)
                            rc = awork.tile([P, 512], F32, tag="recip")
                            nc.vector.reciprocal(rc[HD:HD + 1, :],
                                                 pso[HD:HD + 1, :])
                            rc0 = awork.tile([1, 512], F32, tag="rc0")
                            nc.sync.dma_start(rc0[:], rc[HD:HD + 1, :])
                            rbc = awork.tile([HD, 512], F32, tag="rbc")
                            nc.gpsimd.partition_broadcast(
                                rbc[:], rc0[0:1, :], channels=HD)
                            if h % 2 == 0:
                                nc.vector.tensor_mul(
                                    oT[0:HD, h // 2, qs:qs + 512],
                                    pso[0:HD, :], rbc[:])
                            else:
                                osc = awork.tile([HD, 512], FP8, tag="osc")
                                nc.vector.tensor_mul(osc[:], pso[0:HD, :],
                                                     rbc[:])
                                nc.sync.dma_start(
                                    oT[64:128, h // 2, qs:qs + 512], osc[:])

        # ---- MLP weights (DMA overlaps proj/LN2), proj + LN2, MLP ----
        if ablate != "skip_mlp":
         with ExitStack() as s4:
            w_p = s4.enter_context(tc.tile_pool(name="wmlp", bufs=1))
            xnT_p2 = s4.enter_context(tc.tile_pool(name="xnT2", bufs=1))
            h_p = s4.enter_context(tc.tile_pool(name="hT", bufs=2))
            b2_p = s4.enter_context(tc.tile_pool(name="b2p", bufs=1))

            w1_planes = []
            w1h_sb = w_p.tile([P, KS, HID], FP8)
            for q in range(8):
                nc.sync.dma_start(w1h_sb[:, :, q * 384:(q + 1) * 384],
                                  w1h_d[:, :, q * 384:(q + 1) * 384])
            w1_planes.append(w1h_sb)
            if self.w1x2:
                w1l_sb = w_p.tile([P, KS, HID], FP8)
                for q in range(8):
                    nc.sync.dma_start(w1l_sb[:, :, q * 384:(q + 1) * 384],
                                      w1l_d[:, :, q * 384:(q + 1) * 384])
                w1_planes.append(w1l_sb)
            w2_planes = []
            w2h_sb = w_p.tile([P, HS, C], FP8)
            for c3 in range(4):
                nc.sync.dma_start(w2h_sb[:, c3 * 6:c3 * 6 + 6, :],
                                  w2h_d[:, c3 * 6:c3 * 6 + 6, :])
            w2_planes.append(w2h_sb)
            if self.w2x2:
                w2l_sb = w_p.tile([P, HS, C], FP8)
                for c3 in range(4):
                    nc.sync.dma_start(w2l_sb[:, c3 * 6:c3 * 6 + 6, :],
                                      w2l_d[:, c3 * 6:c3 * 6 + 6, :])
                w2_planes.append(w2l_sb)

            if not self.b2_zero:
                b2_bc = b2_p.tile([P, C], F32)
                nc.sync.dma_start(b2_bc[:], b2_d[:].partition_broadcast(P))

            xnT2 = xnT_p2.tile([P, KS, T], FP8)
            # proj + residual + LN2, interleaved per token tile
            with ExitStack() as s3:
                psum_p = s3.enter_context(
                    tc.tile_pool(name="psp", bufs=2, space="PSUM"))
                psum_ln2 = s3.enter_context(
                    tc.tile_pool(name="psln2", bufs=2, space="PSUM"))
                bp_p = s3.enter_context(tc.tile_pool(name="bpp", bufs=1))
                if not self.bproj_zero:
                    bproj_bc = bp_p.tile([P, C], F32)
                    nc.sync.dma_start(bproj_bc[:],
                                      bproj_d[:].partition_broadcast(P))
                for t in range(NT):
                    if ablate != "skip_attn":
                        psp = psum_p.tile([P, C], F32, tag="psp")
                        for (n0, nsz) in ((0, 512), (512, 256)):
                            for c2 in range(3):
                                nc.tensor.matmul(
                                    psp[:, n0:n0 + nsz],
                                    oT[:, c2 * 2:c2 * 2 + 2, t * P:(t + 1) * P],
                                    wproj_sb[:, c2 * 2:c2 * 2 + 2, n0:n0 + nsz],
                                    start=(c2 == 0), stop=(c2 == 2),
                                    perf_mode=DR)
                        nc.vector.tensor_add(x_sb[:, t, :], x_sb[:, t, :],
                                             psp[:])
                        if not self.bproj_zero:
                            nc.vector.tensor_add(x_sb[:, t, :], x_sb[:, t, :],
                                                 bproj_bc[:])
                    self.ln_tile(x_sb[:, t, :], xnT2, t * P, work, psum_ln2,
                                 eps_t, ident16)

            psum_1 = s4.enter_context(
                tc.tile_pool(name="ps1", bufs=2, space="PSUM"))
            psum_2 = s4.enter_context(
                tc.tile_pool(name="ps2", bufs=2, space="PSUM"))

            n1 = 3 * len(w1_planes)
            n2 = 12 * len(w2_planes)
            for tq in range(T // 512):
                t0 = tq * 512
                hT = h_p.tile([P, HS, 512], FP8, tag="hT")
                for hp in range(12):               # hidden-feature pairs
                    ps1 = psum_1.tile([P, 2, 512], F32, tag="ps1")
                    for j in range(2):
                        i = 0
                        for w1p in w1_planes:
                            for c3 in range(3):
                                nc.tensor.matmul(
                                    ps1[:, j, :],
                                    w1p[:, c3 * 2:c3 * 2 + 2,
                                        (hp * 2 + j) * P:(hp * 2 + j + 1) * P],
                                    xnT2[:, c3 * 2:c3 * 2 + 2, t0:t0 + 512],
                                    start=(i == 0), stop=(i == n1 - 1),
                                    perf_mode=DR)
                                i += 1
                    if self.b1_zero:
                        nc.scalar.activation(
                            hT[:, hp * 2:hp * 2 + 2, :].rearrange(
                                "p a b -> p (a b)"),
                            ps1[:].rearrange("p a b -> p (a b)"),
                            AF.Gelu, scale=1.0 / S)
                    else:
                        for j in range(2):
                            nc.scalar.activation(
                                hT[:, hp * 2 + j, :], ps1[:, j, :],
                                AF.Gelu,
                                bias=b1_sb[:, hp * 2 + j:hp * 2 + j + 1],
                                scale=1.0 / S)
                for tt in range(4):
                    tg = tq * 4 + tt
                    ps2 = psum_2.tile([P, C], F32, tag="ps2")
                    for (n0, nsz) in ((0, 512), (512, 256)):
                        i = 0
                        for hp in range(12):
                            for w2p in w2_planes:
                                nc.tensor.matmul(
                                    ps2[:, n0:n0 + nsz],
                                    hT[:, hp * 2:hp * 2 + 2,
                                       tt * P:(tt + 1) * P],
                                    w2p[:, hp * 2:hp * 2 + 2, n0:n0 + nsz],
                                    start=(i == 0), stop=(i == n2 - 1),
                                    perf_mode=DR)
                                i += 1
                    nc.vector.tensor_add(x_sb[:, tg, :], x_sb[:, tg, :],
                                         ps2[:])
                    if not self.b2_zero:
                        nc.vector.tensor_add(x_sb[:, tg, :], x_sb[:, tg, :],
                                             b2_bc[:])
                nc.sync.dma_start(
                    out_d[:].rearrange("(n p) c -> p n c", p=P)[:, tq * 4:tq * 4 + 4, :],
                    x_sb[:, tq * 4:tq * 4 + 4, :])


def _build(b1_zero=False, bv_zero=False, bproj_zero=False, b2_zero=False,
           bqk_zero=False, w1x2=True, w2x2=True):
    nc = bacc.Bacc(None, target_bir_lowering=False, debug=False)

    x_d = nc.dram_tensor("x", [T, C], F32, kind="ExternalInput")
    out_d = nc.dram_tensor("out", [T, C], F32, kind="ExternalOutput")
    wqkv_d = nc.dram_tensor("wqkv", [P, KS, 3 * C], FP8, kind="ExternalInput")
    bqkv_d = nc.dram_tensor("bqkv", [P, 12], F32, kind="ExternalInput")
    bv_d = nc.dram_tensor("bv", [C], F32, kind="ExternalInput")
    wproj_d = nc.dram_tensor("wproj", [P, KS, C], FP8, kind="ExternalInput")
    bproj_d = nc.dram_tensor("bproj", [C], F32, kind="ExternalInput")
    w1h_d = nc.dram_tensor("w1h", [P, KS, HID], FP8, kind="ExternalInput")
    w1l_d = nc.dram_tensor("w1l", [P, KS, HID], FP8, kind="ExternalInput")
    b1_d = nc.dram_tensor("b1", [P, HS], F32, kind="ExternalInput")
    w2h_d = nc.dram_tensor("w2h", [P, HS, C], FP8, kind="ExternalInput")
    w2l_d = nc.dram_tensor("w2l", [P, HS, C], FP8, kind="ExternalInput")
    b2_d = nc.dram_tensor("b2", [C], F32, kind="ExternalInput")
    with TileKernel(nc) as tk:
        tk.b1_zero = b1_zero
        tk.bqk_zero = bqk_zero
        tk.bv_zero = bv_zero
        tk.bproj_zero = bproj_zero
        tk.b2_zero = b2_zero
        tk.w1x2 = w1x2
        tk.w2x2 = w2x2
        tk.run(x_d, out_d, wqkv_d, bqkv_d, bv_d, wproj_d, bproj_d,
               w1h_d, w1l_d, b1_d, w2h_d, w2l_d, b2_d)

    nc.compile()
    return nc


def _hilo(w):
    hi = w.astype(NP8)
    lo = (w - hi.astype(np.float32)).astype(NP8)
    return hi, lo


def _prep_host(inputs):
    f = lambda a: np.asarray(a, dtype=np.float32)
    x = f(inputs["x"])
    ln1_g, ln1_b = f(inputs["ln1_g"]), f(inputs["ln1_b"])
    ln2_g, ln2_b = f(inputs["ln2_g"]), f(inputs["ln2_b"])
    qkv_w = f(inputs["qkv_w"])
    proj_w, proj_b = f(inputs["proj_w"]), f(inputs["proj_b"])
    fc1_w, fc1_b = f(inputs["fc1_w"]), f(inputs["fc1_b"])
    fc2_w, fc2_b = f(inputs["fc2_w"]), f(inputs["fc2_b"])

    wqkv = np.ascontiguousarray(
        (qkv_w * ln1_g[None, :] * S).T.reshape(KS, P, 3 * C).transpose(1, 0, 2)
    ).astype(NP8)
    bqkv_full = S * (qkv_w @ ln1_b)                # [2304], S-scaled
    bqkv = np.ascontiguousarray(bqkv_full[:2 * C].reshape(12, P).T)
    bv = np.ascontiguousarray(bqkv_full[2 * C:])
    wproj = np.ascontiguousarray(
        (proj_w * S).T.reshape(KS, P, C).transpose(1, 0, 2)).astype(NP8)
    w1 = np.ascontiguousarray(
        (fc1_w * ln2_g[None, :] * S).T.reshape(KS, P, HID).transpose(1, 0, 2))
    w1h, w1l = _hilo(w1)
    b1 = np.ascontiguousarray((fc1_b + fc1_w @ ln2_b).reshape(HS, P).T)
    w2 = np.ascontiguousarray(
        (fc2_w * S).T.reshape(HS, P, C).transpose(1, 0, 2))
    w2h, w2l = _hilo(w2)

    shared = {
        "wqkv": wqkv, "bqkv": bqkv, "bv": bv,
        "wproj": wproj, "bproj": S * proj_b,
        "w1h": w1h, "w1l": w1l, "b1": b1,
        "w2h": w2h, "w2l": w2l, "b2": S * fc2_b,
    }
    in_maps = []
    for c in range(8):
        m = dict(shared)
        m["x"] = np.ascontiguousarray(
            S * x[c * B_PER_CORE:(c + 1) * B_PER_CORE].reshape(T, C))
        in_maps.append(m)
    return in_maps


def kernel(**inputs):
    global _CACHED_NC
    b1_host = (np.asarray(inputs["fc1_b"], np.float32)
               + np.asarray(inputs["fc1_w"], np.float32)
               @ np.asarray(inputs["ln2_b"], np.float32))
    b1_zero = bool(np.all(b1_host == 0.0))
    bqkv_host = (np.asarray(inputs["qkv_w"], np.float32)
                 @ np.asarray(inputs["ln1_b"], np.float32))
    bv_zero = bool(np.all(bqkv_host[2 * C:] == 0.0))
    bqk_zero = bool(np.all(bqkv_host[:2 * C] == 0.0))
    bproj_zero = bool(np.all(np.asarray(inputs["proj_b"]) == 0.0))
    b2_zero = bool(np.all(np.asarray(inputs["fc2_b"]) == 0.0))
    key = (b1_zero, bv_zero, bproj_zero, b2_zero, bqk_zero)
    if _CACHED_NC is None or getattr(_CACHED_NC, "_spec", None) != key:
        _CACHED_NC = _build(b1_zero=b1_zero, bv_zero=bv_zero,
                            bproj_zero=bproj_zero, b2_zero=b2_zero,
                            bqk_zero=bqk_zero)
        _CACHED_NC._spec = key
    nc = _CACHED_NC
    in_maps = _prep_host(inputs)
    trace = os.environ.get("TRN_KERNEL_TRACE", "0") == "1"
    res = run_bass_kernel_spmd(nc, in_maps, core_ids=list(range(8)),
                               trace=trace)
    if trace and res.exec_time_ns is not None:
        print(f"HW exec time: {res.exec_time_ns} ns")
        print(f"mean exec time: {res.mean_exec_time_ns} ns")
        if res.instructions_and_trace is not None:
            print(f"trace: {res.instructions_and_trace[1]}")
    out = np.stack([
        res.results[c]["out"].reshape(B_PER_CORE, SEQ, C) for c in range(8)
    ]).reshape(16, SEQ, C)
    return (out / S).astype(np.float32)
